# revision 21
# baseline (speedup 1.0000x reference)
"""Multi-head attention TRN2 kernel, head-parallel across 8 NeuronCores.

Per core c (= head h=c), all matmuls in float32r (11-bit mantissa, full PE
rate at N=512), keys-on-partitions score layout, with both outer
projections algebraically fused into the K / V projections:

  scores = q Wq (k Wk)^T = q G k^T          G = Wq Wk^T   (host)
  out    = attn (v Wv) Wo = attn (v U)      U = Wv Wo_h   (host)

so the device only computes, per core:

  K2T[d,t] = A k^T   with A = G^T = Wk Wq^T  (lhsT = A nat, rhs = kT)
  V2[t,o]  = v U                             (lhsT = vT,  rhs = U)
  scoresT[t,s] = K2 q^T                      (lhsT = K2T, rhs = qT chunk)
  E = exp(scoresT*scale + Madd + wbias[t])   (DVE mask-add, ACT exp)
  rowsum broadcast via ones[128,128] lhsT matmuls over E tiles
  outT[o,s] = V2^T E / rowsum                (lhsT = V2, rhs = E)

Host: transposes q/k/v, pre-rounds f32r inputs (RNE drop-12, bit-exact vs
HW cast), builds additive bf16 mask (0 / -1e9) in [t,s] orientation,
folds all biases exactly (bk drops under softmax; bq -> per-key exp
bias; bv,bo -> final add), sums per-head partial outputs and transposes
the [dout, s] device layout back to [b, s, dout].
"""
import sys
import numpy as np

sys.path.insert(0, "/opt/trn_rl_repo")

H, D, B, S = 8, 512, 2, 2048
P = 128
NE = D // P            # 4 feature tiles
NT = S // P            # 16 key tiles per batch
CH = 512               # query/key chunk width
NCH = S // CH          # 4 chunks per batch
SCALE = 1.0 / np.sqrt(np.float32(D))

_CACHE = {}


def _f32r_round(x):
    """Bit-exact host emulation of HW fp32->f32r cast (RNE, drop 12 bits)."""
    u = np.ascontiguousarray(x, np.float32).view(np.uint32).astype(np.uint64)
    half = np.uint64(1 << 11)
    lsb = (u >> np.uint64(12)) & np.uint64(1)
    u2 = (u + half - np.uint64(1) + lsb) >> np.uint64(12) << np.uint64(12)
    return u2.astype(np.uint32).view(np.float32).reshape(x.shape)


def _build():
    from contextlib import ExitStack
    from concourse import bass, bacc, tile

    mybir = bass.mybir
    dt = mybir.dt
    AF = mybir.ActivationFunctionType
    ALU = mybir.AluOpType

    nc = bacc.Bacc("TRN2", target_bir_lowering=False, debug=False)

    qT_d = nc.dram_tensor("qT", [D, B * S], dt.float32r, kind="ExternalInput")
    kT_d = nc.dram_tensor("kT", [D, B * S], dt.float32r, kind="ExternalInput")
    vT_d = nc.dram_tensor("vT", [D, B * S], dt.float32r, kind="ExternalInput")
    mT_d = nc.dram_tensor("mT", [S, S], dt.bfloat16, kind="ExternalInput")
    ka_d = nc.dram_tensor("ka", [D, D], dt.float32r, kind="ExternalInput")  # Wk Wq^T
    vu_d = nc.dram_tensor("vu", [D, D], dt.float32r, kind="ExternalInput")  # Wv Wo_h
    wb_d = nc.dram_tensor("wb", [P, B * NT], dt.float32, kind="ExternalInput")
    out_d = nc.dram_tensor("out", [D, B * S], dt.float32, kind="ExternalOutput")
    rs_d = nc.dram_tensor("rs", [P, B * S], dt.float32, kind="ExternalOutput")

    def dtiles(ap_2d):
        return ap_2d.rearrange("(a p) c -> p a c", p=P)

    with tile.TileContext(nc) as tc:
        with ExitStack() as ctx:
            wpool = ctx.enter_context(tc.tile_pool(name="w", bufs=1))
            kvpool = ctx.enter_context(tc.tile_pool(name="kv", bufs=1))
            xin = ctx.enter_context(tc.tile_pool(name="xin", bufs=4))
            epool = ctx.enter_context(tc.tile_pool(name="e", bufs=1))
            mpool = ctx.enter_context(tc.tile_pool(name="m", bufs=2))
            tpool = ctx.enter_context(tc.tile_pool(name="tmp", bufs=3))
            rpool = ctx.enter_context(tc.tile_pool(name="r", bufs=1))
            opool = ctx.enter_context(tc.tile_pool(name="o", bufs=3))
            psA = ctx.enter_context(tc.tile_pool(name="psA", bufs=4, space="PSUM"))
            psO = ctx.enter_context(tc.tile_pool(name="psO", bufs=4, space="PSUM"))

            ka = wpool.tile([P, NE, D], dt.float32r)
            vu = wpool.tile([P, NE, D], dt.float32r)
            nc.sync.dma_start(ka[:], dtiles(ka_d.ap()))
            wb = wpool.tile([P, B * NT], dt.float32)


            K2T = kvpool.tile([P, NE, S], dt.float32r, tag="K2T")
            V2 = kvpool.tile([P, NT, D], dt.float32r, tag="V2")

            qTt = dtiles(qT_d.ap())
            kTt = dtiles(kT_d.ap())
            vTt = dtiles(vT_d.ap())
            mTt = mT_d.ap().rearrange("(a p) c -> p a c", p=P)  # [128, NT, S]

            for b in range(B):
                # ---- stage A: K2^T first (scores-critical), then V2 ----
                qin0 = None
                for tc4 in range(NCH):
                    col0 = b * S + tc4 * CH
                    kin = xin.tile([P, NE, CH], dt.float32r, tag="xin")
                    nc.sync.dma_start(kin[:], kTt[:, :, col0:col0 + CH])
                    if tc4 == 2:
                        qin0 = xin.tile([P, NE, CH], dt.float32r, tag="xin")
                        nc.sync.dma_start(qin0[:], qTt[:, :, b * S:b * S + CH])
                    for et in range(NE):
                        ps = psA.tile([P, CH], dt.float32, tag="ps")
                        for kd in range(NE):
                            nc.tensor.matmul(
                                ps[:], ka[:, kd, et * P:(et + 1) * P], kin[:, kd, :],
                                start=(kd == 0), stop=(kd == NE - 1))
                        nc.scalar.copy(K2T[:, et, tc4 * CH:(tc4 + 1) * CH], ps[:])
                if b == 0:
                    nc.sync.dma_start(vu[:], dtiles(vu_d.ap()))
                    nc.sync.dma_start(wb[:], wb_d[:])
                for tc4 in range(NCH):
                    col0 = b * S + tc4 * CH
                    vin = xin.tile([P, NE, CH], dt.float32r, tag="xin")
                    nc.sync.dma_start(vin[:], vTt[:, :, col0:col0 + CH])
                    for ts in range(CH // P):
                        ps = psA.tile([P, D], dt.float32, tag="ps")
                        for kd in range(NE):
                            nc.tensor.matmul(
                                ps[:], vin[:, kd, ts * P:(ts + 1) * P], vu[:, kd, :],
                                start=(kd == 0), stop=(kd == NE - 1))
                        nc.scalar.copy(V2[:, tc4 * (CH // P) + ts, :], ps[:])

                # ---- stage B: per query-chunk attention ----
                for c in range(NCH):
                    col0 = b * S + c * CH
                    if c == 0:
                        qin = qin0
                    else:
                        qin = xin.tile([P, NE, CH], dt.float32r, tag="xin")
                        nc.sync.dma_start(qin[:], qTt[:, :, col0:col0 + CH])
                    mt = mpool.tile([P, NT, CH], dt.bfloat16)
                    nc.gpsimd.dma_start(mt[:], mTt[:, :, c * CH:(c + 1) * CH])

                    E = epool.tile([P, NT, CH], dt.float32r)
                    for tt in range(NT):
                        ps = psA.tile([P, CH], dt.float32, tag="ps")
                        for et in range(NE):
                            nc.tensor.matmul(
                                ps[:], K2T[:, et, tt * P:(tt + 1) * P], qin[:, et, :],
                                start=(et == 0), stop=(et == NE - 1))
                        tmp = tpool.tile([P, CH], dt.float32)
                        nc.vector.scalar_tensor_tensor(
                            tmp[:], ps[:], float(SCALE), mt[:, tt, :],
                            op0=ALU.mult, op1=ALU.add)
                        nc.scalar.activation(
                            E[:, tt, :], tmp[:], AF.Exp,
                            bias=wb[:, b * NT + tt: b * NT + tt + 1], scale=1.0)

                    Ef = E[:].bitcast(dt.float32)
                    red = rpool.tile([P, NT // 2, CH], dt.float32, tag="red")
                    nc.vector.tensor_add(red[:], Ef[:, 0:8, :], Ef[:, 8:16, :])
                    nc.vector.tensor_add(red[:, 0:4, :], red[:, 0:4, :], red[:, 4:8, :])
                    nc.vector.tensor_add(red[:, 0:2, :], red[:, 0:2, :], red[:, 2:4, :])
                    accr = rpool.tile([P, CH], dt.float32, tag="accr")
                    nc.vector.tensor_add(accr[:], red[:, 0, :], red[:, 1, :])
                    nc.gpsimd.dma_start(rs_d[:, col0:col0 + CH], accr[:])

                    pso = [psO.tile([P, CH], dt.float32, tag="pso", name=f"pso{i}") for i in range(NE)]
                    for tt in range(NT):
                        for os_ in range(NE):
                            nc.tensor.matmul(
                                pso[os_][:], V2[:, tt, os_ * P:(os_ + 1) * P],
                                E[:, tt, :],
                                start=(tt == 0), stop=(tt == NT - 1))
                    for os_ in range(NE):
                        ot = opool.tile([P, CH], dt.float32)
                        nc.vector.tensor_copy(ot[:], pso[os_][:])
                        r0 = os_ * P
                        nc.gpsimd.dma_start(out_d[r0:r0 + P, col0:col0 + CH], ot[:])

    nc.compile()
    return nc


def kernel(q, k, v, mask, Wq, bq, Wk, bk, Wv, bv, Wo, bo):
    from concourse.bass_utils import run_bass_kernel_spmd
    import ml_dtypes

    q = np.asarray(q, np.float32)
    k = np.asarray(k, np.float32)
    v = np.asarray(v, np.float32)
    mask = np.asarray(mask)
    Wq = np.asarray(Wq, np.float32)
    Wk = np.asarray(Wk, np.float32)
    Wv = np.asarray(Wv, np.float32)
    Wo = np.asarray(Wo, np.float32)
    bq = np.asarray(bq, np.float32)
    bk = np.asarray(bk, np.float32)
    bv = np.asarray(bv, np.float32)
    bo = np.asarray(bo, np.float32)

    qT = _f32r_round(q.transpose(2, 0, 1).reshape(D, B * S))
    kT = _f32r_round(k.transpose(2, 0, 1).reshape(D, B * S))
    vT = _f32r_round(v.transpose(2, 0, 1).reshape(D, B * S))
    mT = np.where(mask.T == 1, np.float32(-1e9), np.float32(0.0)).astype(ml_dtypes.bfloat16)
    mT = np.ascontiguousarray(mT)

    kf = k.reshape(B * S, D)
    in_maps = []
    for h in range(H):
        Wq64 = Wq[h].astype(np.float64)
        Wk64 = Wk[h].astype(np.float64)
        Wv64 = Wv[h].astype(np.float64)
        Wo64 = Wo[h * D:(h + 1) * D, :].astype(np.float64)
        A = (Wk64 @ Wq64.T).astype(np.float32)       # lhsT for K2^T proj
        U = (Wv64 @ Wo64).astype(np.float32)         # rhs for V2 proj
        wvec = (kf @ (Wk[h] @ bq[h])) * SCALE        # per-key exp bias
        wb = np.ascontiguousarray(wvec.reshape(B * NT, P).T.astype(np.float32))
        in_maps.append({
            "qT": qT, "kT": kT, "vT": vT, "mT": mT,
            "ka": _f32r_round(A), "vu": _f32r_round(U), "wb": wb,
        })

    if "nc" not in _CACHE:
        _CACHE["nc"] = _build()
    nc = _CACHE["nc"]
    _CACHE["in_maps"] = in_maps

    res = run_bass_kernel_spmd(nc, in_maps, core_ids=list(range(H)))
    total = np.zeros((D, B * S), np.float64)
    for h in range(H):
        r = res.results[h]["rs"].sum(axis=0, dtype=np.float64)   # [B*S]
        total += res.results[h]["out"].astype(np.float64) / r[None, :]

    cvec = bo.astype(np.float64).copy()
    for h in range(H):
        cvec += bv[h].astype(np.float64) @ Wo[h * D:(h + 1) * D, :].astype(np.float64)
    total += cvec[:, None]
    return total.T.astype(np.float32).reshape(B, S, D)


# revision 22
# speedup vs baseline: 1.0135x; 1.0135x over previous
"""Multi-head attention TRN2 kernel, head-parallel across 8 NeuronCores.

Per core c (= head h=c), all matmuls in float32r (11-bit mantissa, full PE
rate at N=512), keys-on-partitions score layout, with both outer
projections algebraically fused into the K / V projections:

  scores = q Wq (k Wk)^T = q G k^T          G = Wq Wk^T   (host)
  out    = attn (v Wv) Wo = attn (v U)      U = Wv Wo_h   (host)

so the device only computes, per core:

  K2T[d,t] = A k^T   with A = G^T = Wk Wq^T  (lhsT = A nat, rhs = kT)
  V2[t,o]  = v U                             (lhsT = vT,  rhs = U)
  scoresT[t,s] = K2 q^T                      (lhsT = K2T, rhs = qT chunk)
  E = exp(scoresT*scale + Madd + wbias[t])   (DVE mask-add, ACT exp)
  rowsum broadcast via ones[128,128] lhsT matmuls over E tiles
  outT[o,s] = V2^T E / rowsum                (lhsT = V2, rhs = E)

Host: transposes q/k/v, pre-rounds f32r inputs (RNE drop-12, bit-exact vs
HW cast), builds additive bf16 mask (0 / -1e9) in [t,s] orientation,
folds all biases exactly (bk drops under softmax; bq -> per-key exp
bias; bv,bo -> final add), sums per-head partial outputs and transposes
the [dout, s] device layout back to [b, s, dout].
"""
import sys
import numpy as np

sys.path.insert(0, "/opt/trn_rl_repo")

H, D, B, S = 8, 512, 2, 2048
P = 128
NE = D // P            # 4 feature tiles
NT = S // P            # 16 key tiles per batch
CH = 512               # query/key chunk width
NCH = S // CH          # 4 chunks per batch
SCALE = 1.0 / np.sqrt(np.float32(D))

_CACHE = {}


def _f32r_round(x):
    """Bit-exact host emulation of HW fp32->f32r cast (RNE, drop 12 bits)."""
    u = np.ascontiguousarray(x, np.float32).view(np.uint32).astype(np.uint64)
    half = np.uint64(1 << 11)
    lsb = (u >> np.uint64(12)) & np.uint64(1)
    u2 = (u + half - np.uint64(1) + lsb) >> np.uint64(12) << np.uint64(12)
    return u2.astype(np.uint32).view(np.float32).reshape(x.shape)


def _build():
    from contextlib import ExitStack
    from concourse import bass, bacc, tile

    mybir = bass.mybir
    dt = mybir.dt
    AF = mybir.ActivationFunctionType
    ALU = mybir.AluOpType

    nc = bacc.Bacc("TRN2", target_bir_lowering=False, debug=False)

    qT_d = nc.dram_tensor("qT", [D, B * S], dt.float32r, kind="ExternalInput")
    kT_d = nc.dram_tensor("kT", [D, B * S], dt.float32r, kind="ExternalInput")
    vT_d = nc.dram_tensor("vT", [D, B * S], dt.float32r, kind="ExternalInput")
    mT_d = nc.dram_tensor("mT", [S, S], dt.bfloat16, kind="ExternalInput")
    ka_d = nc.dram_tensor("ka", [D, D], dt.float32r, kind="ExternalInput")  # Wk Wq^T
    vu_d = nc.dram_tensor("vu", [D, D], dt.float32r, kind="ExternalInput")  # Wv Wo_h
    wb_d = nc.dram_tensor("wb", [P, B * NT], dt.float32, kind="ExternalInput")
    out_d = nc.dram_tensor("out", [D, B * S], dt.float32, kind="ExternalOutput")
    rs_d = nc.dram_tensor("rs", [P, B * S], dt.float32, kind="ExternalOutput")

    def dtiles(ap_2d):
        return ap_2d.rearrange("(a p) c -> p a c", p=P)

    with tile.TileContext(nc) as tc:
        with ExitStack() as ctx:
            wpool = ctx.enter_context(tc.tile_pool(name="w", bufs=1))
            kvpool = ctx.enter_context(tc.tile_pool(name="kv", bufs=1))
            xin = ctx.enter_context(tc.tile_pool(name="xin", bufs=4))
            epool = ctx.enter_context(tc.tile_pool(name="e", bufs=1))
            mpool = ctx.enter_context(tc.tile_pool(name="m", bufs=2))
            tpool = ctx.enter_context(tc.tile_pool(name="tmp", bufs=3))
            rpool = ctx.enter_context(tc.tile_pool(name="r", bufs=1))
            opool = ctx.enter_context(tc.tile_pool(name="o", bufs=3))
            psA = ctx.enter_context(tc.tile_pool(name="psA", bufs=4, space="PSUM"))
            psO = ctx.enter_context(tc.tile_pool(name="psO", bufs=4, space="PSUM"))

            ka = wpool.tile([P, NE, D], dt.float32r)
            vu = wpool.tile([P, NE, D], dt.float32r)
            nc.sync.dma_start(ka[:], dtiles(ka_d.ap()))
            wb = wpool.tile([P, B * NT], dt.float32)


            K2T = kvpool.tile([P, NE, S], dt.float32r, tag="K2T")
            V2 = kvpool.tile([P, NT, D], dt.float32r, tag="V2")

            qTt = dtiles(qT_d.ap())
            kTt = dtiles(kT_d.ap())
            vTt = dtiles(vT_d.ap())
            mTt = mT_d.ap().rearrange("(a p) c -> p a c", p=P)  # [128, NT, S]

            for b in range(B):
                # ---- stage A: K2^T first (scores-critical), then V2 ----
                qin0 = None
                for tc4 in range(NCH):
                    col0 = b * S + tc4 * CH
                    kin = xin.tile([P, NE, CH], dt.float32r, tag="xin")
                    nc.sync.dma_start(kin[:], kTt[:, :, col0:col0 + CH])
                    if tc4 == 2:
                        qin0 = xin.tile([P, NE, CH], dt.float32r, tag="xin")
                        nc.sync.dma_start(qin0[:], qTt[:, :, b * S:b * S + CH])
                    for et in range(NE):
                        ps = psA.tile([P, CH], dt.float32, tag="ps")
                        for kd in range(NE):
                            nc.tensor.matmul(
                                ps[:], ka[:, kd, et * P:(et + 1) * P], kin[:, kd, :],
                                start=(kd == 0), stop=(kd == NE - 1))
                        nc.scalar.copy(K2T[:, et, tc4 * CH:(tc4 + 1) * CH], ps[:])
                if b == 0:
                    nc.sync.dma_start(vu[:], dtiles(vu_d.ap()))
                    nc.sync.dma_start(wb[:], wb_d[:])
                for tc4 in range(NCH):
                    col0 = b * S + tc4 * CH
                    vin = xin.tile([P, NE, CH], dt.float32r, tag="xin")
                    nc.sync.dma_start(vin[:], vTt[:, :, col0:col0 + CH])
                    for ts in range(CH // P):
                        ps = psA.tile([P, D], dt.float32, tag="ps")
                        for kd in range(NE):
                            nc.tensor.matmul(
                                ps[:], vin[:, kd, ts * P:(ts + 1) * P], vu[:, kd, :],
                                start=(kd == 0), stop=(kd == NE - 1))
                        nc.scalar.copy(V2[:, tc4 * (CH // P) + ts, :], ps[:])

                # ---- stage B: per query-chunk attention ----
                for c in range(NCH):
                    col0 = b * S + c * CH
                    if c == 0:
                        qin = qin0
                    else:
                        qin = xin.tile([P, NE, CH], dt.float32r, tag="xin")
                        nc.sync.dma_start(qin[:], qTt[:, :, col0:col0 + CH])
                    mt = mpool.tile([P, NT, CH], dt.bfloat16)
                    nc.gpsimd.dma_start(mt[:], mTt[:, :, c * CH:(c + 1) * CH])

                    E = epool.tile([P, NT, CH], dt.float32r)
                    for tt in range(NT):
                        ps = psA.tile([P, CH], dt.float32, tag="ps")
                        for et in range(NE):
                            nc.tensor.matmul(
                                ps[:], K2T[:, et, tt * P:(tt + 1) * P], qin[:, et, :],
                                start=(et == 0), stop=(et == NE - 1))
                        tmp = tpool.tile([P, CH], dt.float32)
                        nc.vector.scalar_tensor_tensor(
                            tmp[:], ps[:], float(SCALE), mt[:, tt, :],
                            op0=ALU.mult, op1=ALU.add)
                        nc.scalar.activation(
                            E[:, tt, :], tmp[:], AF.Exp,
                            bias=wb[:, b * NT + tt: b * NT + tt + 1], scale=1.0)

                    Ef = E[:].bitcast(dt.float32)
                    red = rpool.tile([P, NT // 2, CH], dt.float32, tag="red")
                    nc.vector.tensor_add(red[:], Ef[:, 0:8, :], Ef[:, 8:16, :])
                    nc.vector.tensor_add(red[:, 0:4, :], red[:, 0:4, :], red[:, 4:8, :])
                    nc.vector.tensor_add(red[:, 0:2, :], red[:, 0:2, :], red[:, 2:4, :])
                    accr = rpool.tile([P, CH], dt.float32, tag="accr")
                    nc.vector.tensor_add(accr[:], red[:, 0, :], red[:, 1, :])
                    nc.gpsimd.dma_start(rs_d[:, col0:col0 + CH], accr[:])

                    pso = [psO.tile([P, CH], dt.float32, tag="pso", name=f"pso{i}") for i in range(NE)]
                    for tt in range(NT):
                        for os_ in range(NE):
                            nc.tensor.matmul(
                                pso[os_][:], V2[:, tt, os_ * P:(os_ + 1) * P],
                                E[:, tt, :],
                                start=(tt == 0), stop=(tt == NT - 1))
                    for os_ in range(NE):
                        ot = opool.tile([P, CH], dt.float32)
                        nc.scalar.copy(ot[:], pso[os_][:])
                        r0 = os_ * P
                        nc.gpsimd.dma_start(out_d[r0:r0 + P, col0:col0 + CH], ot[:])

    nc.compile()
    return nc


def kernel(q, k, v, mask, Wq, bq, Wk, bk, Wv, bv, Wo, bo):
    from concourse.bass_utils import run_bass_kernel_spmd
    import ml_dtypes

    q = np.asarray(q, np.float32)
    k = np.asarray(k, np.float32)
    v = np.asarray(v, np.float32)
    mask = np.asarray(mask)
    Wq = np.asarray(Wq, np.float32)
    Wk = np.asarray(Wk, np.float32)
    Wv = np.asarray(Wv, np.float32)
    Wo = np.asarray(Wo, np.float32)
    bq = np.asarray(bq, np.float32)
    bk = np.asarray(bk, np.float32)
    bv = np.asarray(bv, np.float32)
    bo = np.asarray(bo, np.float32)

    qT = _f32r_round(q.transpose(2, 0, 1).reshape(D, B * S))
    kT = _f32r_round(k.transpose(2, 0, 1).reshape(D, B * S))
    vT = _f32r_round(v.transpose(2, 0, 1).reshape(D, B * S))
    mT = np.where(mask.T == 1, np.float32(-1e9), np.float32(0.0)).astype(ml_dtypes.bfloat16)
    mT = np.ascontiguousarray(mT)

    kf = k.reshape(B * S, D)
    in_maps = []
    for h in range(H):
        Wq64 = Wq[h].astype(np.float64)
        Wk64 = Wk[h].astype(np.float64)
        Wv64 = Wv[h].astype(np.float64)
        Wo64 = Wo[h * D:(h + 1) * D, :].astype(np.float64)
        A = (Wk64 @ Wq64.T).astype(np.float32)       # lhsT for K2^T proj
        U = (Wv64 @ Wo64).astype(np.float32)         # rhs for V2 proj
        wvec = (kf @ (Wk[h] @ bq[h])) * SCALE        # per-key exp bias
        wb = np.ascontiguousarray(wvec.reshape(B * NT, P).T.astype(np.float32))
        in_maps.append({
            "qT": qT, "kT": kT, "vT": vT, "mT": mT,
            "ka": _f32r_round(A), "vu": _f32r_round(U), "wb": wb,
        })

    if "nc" not in _CACHE:
        _CACHE["nc"] = _build()
    nc = _CACHE["nc"]
    _CACHE["in_maps"] = in_maps

    res = run_bass_kernel_spmd(nc, in_maps, core_ids=list(range(H)))
    total = np.zeros((D, B * S), np.float64)
    for h in range(H):
        r = res.results[h]["rs"].sum(axis=0, dtype=np.float64)   # [B*S]
        total += res.results[h]["out"].astype(np.float64) / r[None, :]

    cvec = bo.astype(np.float64).copy()
    for h in range(H):
        cvec += bv[h].astype(np.float64) @ Wo[h * D:(h + 1) * D, :].astype(np.float64)
    total += cvec[:, None]
    return total.T.astype(np.float32).reshape(B, S, D)
